# revision 15
# baseline (speedup 1.0000x reference)
"""Trainium2 Bass kernel for nn_MaskedSelfAttention (causal, QK rms-norm).

Sharding: 8 cores = 2 (batch) x 4 (head groups of 4 heads).
Each core computes qkv projection for its heads, causal flash-style
attention (no max subtraction -- scores are bounded by rms norm), and a
partial FC output over its heads' feature slice. Host sums the 4 partials
per batch.

Self-contained: hardcodes shapes from the problem spec.
"""

import numpy as np

import concourse.bacc as bacc
import concourse.mybir as mybir
import concourse.tile as tile
from concourse.bass_utils import run_bass_kernel_spmd

B, L, D = 2, 2048, 1024
DH = 64
NH = D // DH            # 16 heads total
P = 128
NHC = 4                 # heads per core
E3 = 3 * NHC * DH       # 768 qkv rows per core
LB = L // P             # 16 l-blocks
KB = D // P             # 8 contraction blocks
EPS = 1e-5
F32 = mybir.dt.float32
FX = mybir.ActivationFunctionType
MULT = mybir.AluOpType.mult

_CACHE = {}


def _build_nc():
    nc = bacc.Bacc("TRN2", target_bir_lowering=False, debug=False)

    xT = nc.dram_tensor("xT", (D, L), F32, kind="ExternalInput").ap()
    wqkvT = nc.dram_tensor("wqkvT", (D, E3), F32, kind="ExternalInput").ap()
    wfcT = nc.dram_tensor("wfcT", (NHC * DH, D), F32, kind="ExternalInput").ap()
    triu = nc.dram_tensor("triu", (P, P), F32, kind="ExternalInput").ap()
    wqk = nc.dram_tensor("wqk", (P, 1), F32, kind="ExternalInput").ap()
    ident = nc.dram_tensor("ident", (P, P), F32, kind="ExternalInput").ap()
    sel = nc.dram_tensor("sel", (NHC, NHC * DH), F32, kind="ExternalInput").ap()
    outp = nc.dram_tensor("outp", (L, D), F32, kind="ExternalOutput").ap()

    with tile.TileContext(nc) as tc:
        with (
            tc.tile_pool(name="cpool", bufs=1) as cpool,
            tc.tile_pool(name="wpool", bufs=1) as wpool,
            tc.tile_pool(name="ppool", bufs=1) as ppool,
            tc.tile_pool(name="xpool", bufs=16) as xpool,
            tc.tile_pool(name="work", bufs=4) as work,
            tc.tile_pool(name="ptpool", bufs=3) as ptpool,
            tc.tile_pool(name="opool", bufs=3) as opool,
        ):
            ident_sb = cpool.tile([P, P], F32)
            nc.sync.dma_start(ident_sb, ident)
            triu_sb = cpool.tile([P, P], F32)
            nc.sync.dma_start(triu_sb, triu)
            wqk_sb = cpool.tile([P, 1], F32)
            nc.sync.dma_start(wqk_sb, wqk)
            sel_sb = cpool.tile([NHC, NHC * DH], F32)
            nc.sync.dma_start(sel_sb, sel)
            biasq = cpool.tile([P, 1], F32)
            nc.vector.memset(biasq, DH * EPS)
            biask = cpool.tile([P, 1], F32)
            nc.vector.memset(biask, EPS)

            wqkv_sb = wpool.tile([P, KB, E3], F32)
            nc.sync.dma_start(wqkv_sb, wqkvT.rearrange("(ko p) e -> p ko e", p=P))
            wfc_sb = wpool.tile([P, 2, D], F32)
            nc.sync.dma_start(wfc_sb, wfcT.rearrange("(g p) e -> p g e", p=P))

            # persistent activations (per-partition fp32 bytes in comments)
            qT = ppool.tile([P, 2, L], F32)        # 16KB  [dh-pair, hp, l]
            kT = ppool.tile([P, 2, L], F32)        # 16KB
            vext = ppool.tile([P, LB, NHC, DH + 1], F32)  # 16.25KB, col DH = ones
            oT = ppool.tile([P, 2, L], F32)        # 16KB  unnorm O^T, normed in place
            rec = ppool.tile([NHC, L], F32)        # 1/denom, free-major
            dnT = ppool.tile([P, LB, NHC], F32)    # denom, lq-partition-major
            recT = ppool.tile([P, LB, NHC], F32)

            nc.vector.memset(vext[:, :, :, DH : DH + 1], 1.0)

            # ---- Phase A: qkv projection (l,e') + rms norm + transpose q,k ----
            with (
                tc.tile_pool(name="psA", bufs=6, space="PSUM") as psA,
                tc.tile_pool(name="psT", bufs=2, space="PSUM") as psT,
            ):
                for m in range(LB):
                    xts = []
                    for k in range(KB):
                        xt = xpool.tile([P, P], F32, tag="xt", name=f"xt_{m}_{k}")
                        nc.sync.dma_start(xt, xT[k * P : (k + 1) * P, m * P : (m + 1) * P])
                        xts.append(xt)
                    for ci in range(3):  # 0=q, 1=k, 2=v
                        base = ci * NHC * DH
                        ps = psA.tile([P, NHC * DH], F32, tag="qkv", name=f"qkv_{m}_{ci}")
                        for k in range(KB):
                            nc.tensor.matmul(
                                ps,
                                lhsT=xts[k],
                                rhs=wqkv_sb[:, k, base : base + NHC * DH],
                                start=(k == 0),
                                stop=(k == KB - 1),
                            )
                        if ci == 2:
                            nc.vector.tensor_copy(
                                vext[:, m, :, 0:DH],
                                ps.rearrange("p (h d) -> p h d", d=DH),
                            )
                            continue
                        sq = work.tile([P, NHC * DH], F32, tag="sq", name=f"sq_{m}_{ci}")
                        nc.scalar.activation(sq, ps, FX.Square)
                        ssq = work.tile([P, NHC], F32, tag="ssq", name=f"ssq_{m}_{ci}")
                        nc.vector.reduce_sum(
                            ssq,
                            sq.rearrange("p (h d) -> p h d", d=DH),
                            axis=mybir.AxisListType.X,
                        )
                        rin = work.tile([P, NHC], F32, tag="rin", name=f"rin_{m}_{ci}")
                        if ci == 0:
                            # 1/rin = 0.125 / sqrt(mean + eps)  (folds sdpa scale)
                            nc.scalar.activation(rin, ssq, FX.Sqrt, bias=biasq[:, :], scale=1.0)
                        else:
                            nc.scalar.activation(rin, ssq, FX.Sqrt, bias=biask[:, :], scale=1.0 / DH)
                        inv = work.tile([P, NHC], F32, tag="inv", name=f"inv_{m}_{ci}")
                        nc.vector.reciprocal(inv, rin)
                        qn = work.tile([P, NHC * DH], F32, tag="qn", name=f"qn_{m}_{ci}")
                        nc.vector.tensor_tensor(
                            qn.rearrange("p (h d) -> p h d", d=DH),
                            ps.rearrange("p (h d) -> p h d", d=DH),
                            inv[:, :, None].to_broadcast((P, NHC, DH)),
                            MULT,
                        )
                        dst = qT if ci == 0 else kT
                        for g in range(2):
                            tp = psT.tile([P, P], F32, tag="tp", name=f"tp_{m}_{ci}_{g}")
                            nc.tensor.transpose(tp, qn[:, g * P : (g + 1) * P], ident_sb)
                            nc.vector.tensor_copy(dst[:, g, m * P : (m + 1) * P], tp)

            # fold norm weights (q_norm_w * k_norm_w) into kT, per-partition
            nc.vector.tensor_scalar_mul(kT, kT, wqk_sb)

            # ---- Phase B: attention. S^T = kT.T@qT, P^T = exp, O^T += V^T@P^T ----
            with (
                tc.tile_pool(name="psS", bufs=2, space="PSUM") as psS,
                tc.tile_pool(name="psO", bufs=3, space="PSUM") as psO,
            ):
                for hp in range(2):
                    for c in range(4):
                        oTps = [
                            psO.tile([DH + 1, 512], F32, tag="oT", name=f"oT_{hp}_{c}_{h2}")
                            for h2 in range(2)
                        ]
                        nj = 4 * c + 4
                        for j in range(nj):
                            off = max(0, j * P - c * 512)
                            W = 512 - off
                            st = psS.tile([P, 2, 512], F32, tag="sT", name=f"sT_{hp}_{c}_{j}")
                            for h2 in range(2):
                                nc.tensor.matmul(
                                    st[:, h2, 0:W],
                                    lhsT=kT[h2 * DH : (h2 + 1) * DH, hp, j * P : (j + 1) * P],
                                    rhs=qT[h2 * DH : (h2 + 1) * DH, hp, c * 512 + off : (c + 1) * 512],
                                    start=True,
                                    stop=True,
                                )
                            pt = ptpool.tile([P, 2, 512], F32, tag="pt", name=f"pt_{hp}_{c}_{j}")
                            nc.scalar.activation(pt[:, :, 0:W], st[:, :, 0:W], FX.Exp)
                            if j >= 4 * c:
                                nc.vector.tensor_tensor(
                                    pt[:, :, 0:P],
                                    pt[:, :, 0:P],
                                    triu_sb[:, None, :].to_broadcast((P, 2, P)),
                                    MULT,
                                )
                            for h2 in range(2):
                                nc.tensor.matmul(
                                    oTps[h2][:, off:512],
                                    lhsT=vext[:, j, 2 * hp + h2, :],
                                    rhs=pt[:, h2, 0:W],
                                    start=(j == 0),
                                    stop=(j == nj - 1),
                                    skip_group_check=True,
                                )
                        for h2 in range(2):
                            lh = 2 * hp + h2
                            # stage denom row at partition 64 (no partition shift),
                            # then PE-transpose 128-col pieces to lq-partition-major
                            dnc = work.tile([DH + 1, 512], F32, tag="dnc", name=f"dnc_{hp}_{c}_{h2}")
                            nc.vector.tensor_copy(dnc[DH : DH + 1, :], oTps[h2][DH : DH + 1, :])
                            dnps = psO.tile([P, NHC], F32, tag="dnT", bufs=1, name=f"dnps_{hp}_{c}_{h2}")
                            for mi in range(4):
                                nc.tensor.transpose(
                                    dnps[:, mi : mi + 1],
                                    dnc[DH : DH + 1, mi * P : (mi + 1) * P],
                                    ident_sb[DH : DH + 1, DH : DH + 1],
                                )
                            nc.vector.tensor_copy(dnT[:, 4 * c : 4 * c + 4, lh], dnps)
                            nc.vector.tensor_copy(
                                oT[h2 * DH : (h2 + 1) * DH, hp, c * 512 : (c + 1) * 512],
                                oTps[h2][0:DH, :],
                            )

            # ---- Phase C: reciprocal of denominators + normalize O^T + FC ----
            with (
                tc.tile_pool(name="psC", bufs=2, space="PSUM") as psC,
                tc.tile_pool(name="psR", bufs=2, space="PSUM") as psR,
                tc.tile_pool(name="psF", bufs=2, space="PSUM") as psF,
            ):
                nc.vector.reciprocal(
                    recT.rearrange("p a b -> p (a b)"),
                    dnT.rearrange("p a b -> p (a b)"),
                )
                for m in range(LB):
                    tp2 = psC.tile([NHC, P], F32, tag="recb", name=f"recb_{m}")
                    nc.tensor.transpose(tp2, recT[:, m, :], ident_sb)
                    nc.vector.tensor_copy(rec[:, m * P : (m + 1) * P], tp2)
                for hp in range(2):
                    for h2 in range(2):
                        lh = 2 * hp + h2
                        for c in range(4):
                            rb = psR.tile([DH, 512], F32, tag="rb", name=f"rb_{lh}_{c}")
                            nc.tensor.matmul(
                                rb,
                                lhsT=sel_sb[:, lh * DH : (lh + 1) * DH],
                                rhs=rec[:, c * 512 : (c + 1) * 512],
                                start=True,
                                stop=True,
                            )
                            seg = oT[h2 * DH : (h2 + 1) * DH, hp, c * 512 : (c + 1) * 512]
                            nc.vector.tensor_tensor(seg, seg, rb, MULT)
                for m in range(LB):
                    for n in range(2):
                        fp = psF.tile([P, 512], F32, tag="fc", name=f"fc_{m}_{n}")
                        for g in range(2):
                            nc.tensor.matmul(
                                fp,
                                lhsT=oT[:, g, m * P : (m + 1) * P],
                                rhs=wfc_sb[:, g, n * 512 : (n + 1) * 512],
                                start=(g == 0),
                                stop=(g == 1),
                            )
                        ot = opool.tile([P, 512], F32, tag="ot", name=f"ot_{m}_{n}")
                        nc.vector.tensor_copy(ot, fp)
                        nc.sync.dma_start(outp[m * P : (m + 1) * P, n * 512 : (n + 1) * 512], ot)

    nc.compile()
    return nc


def _make_in_maps(x, w_qkv, w_fc, q_norm_w, k_norm_w):
    triu_f = np.triu(np.ones((P, P), dtype=np.float32))
    ident = np.eye(P, dtype=np.float32)
    sel = np.kron(np.eye(NHC), np.ones((1, DH))).astype(np.float32)
    wqk = np.tile((q_norm_w * k_norm_w).astype(np.float32), 2).reshape(P, 1)
    wqkvT = {}
    wfcTs = {}
    for hg in range(4):
        h0 = hg * NHC
        rows = np.concatenate(
            [
                w_qkv[h0 * DH : (h0 + NHC) * DH],
                w_qkv[D + h0 * DH : D + (h0 + NHC) * DH],
                w_qkv[2 * D + h0 * DH : 2 * D + (h0 + NHC) * DH],
            ],
            axis=0,
        )
        wqkvT[hg] = np.ascontiguousarray(rows.T.astype(np.float32))
        wfcTs[hg] = np.ascontiguousarray(w_fc.T[h0 * DH : (h0 + NHC) * DH].astype(np.float32))
    xTs = [np.ascontiguousarray(x[b].T.astype(np.float32)) for b in range(B)]
    in_maps = []
    for core in range(8):
        b, hg = core // 4, core % 4
        in_maps.append(
            {
                "xT": xTs[b],
                "wqkvT": wqkvT[hg],
                "wfcT": wfcTs[hg],
                "triu": triu_f,
                "wqk": wqk,
                "ident": ident,
                "sel": sel,
            }
        )
    return in_maps


def _is_causal(mask):
    idx = np.arange(mask.shape[0])
    return mask.shape == (L, L) and bool(np.all(mask == (idx[None, :] <= idx[:, None])))


def _reference_numpy(x, mask, w_qkv, w_fc, q_norm_w, k_norm_w, subset_attention_size):
    # slow but general fallback (only used if mask is not causal)
    b, l, d = x.shape
    qkv = x @ w_qkv.T
    q, k, v = np.split(qkv, 3, axis=-1)

    def heads(t):
        return t.reshape(b, l, NH, DH).transpose(0, 2, 1, 3)

    def rms(t, w):
        return t * (1.0 / np.sqrt(np.mean(t * t, -1, keepdims=True) + EPS)) * w

    q, k, v = heads(q), heads(k), heads(v)
    q, k = rms(q, q_norm_w), rms(k, k_norm_w)

    def sdpa(q, k, v, m):
        s = np.einsum("bhqd,bhkd->bhqk", q, k) / np.sqrt(DH)
        s = np.where(m[None, None], s, -1e30)
        s = s - s.max(-1, keepdims=True)
        p = np.exp(s)
        p /= p.sum(-1, keepdims=True)
        return np.einsum("bhqk,bhkd->bhqd", p, v)

    S = int(subset_attention_size) if subset_attention_size is not None else None
    if S is not None and S < l:
        o = np.concatenate(
            [
                sdpa(q[:, :, :S], k[:, :, :S], v[:, :, :S], mask[:S, :S]),
                sdpa(q[:, :, S:], k, v, mask[S:, :]),
            ],
            axis=2,
        )
    else:
        o = sdpa(q, k, v, mask)
    o = o.transpose(0, 2, 1, 3).reshape(b, l, d)
    return (o @ w_fc.T).astype(np.float32)


def kernel(**inputs):
    x = np.asarray(inputs["x"], dtype=np.float32)
    mask = np.asarray(inputs["mask"])
    w_qkv = np.asarray(inputs["w_qkv"], dtype=np.float32)
    w_fc = np.asarray(inputs["w_fc"], dtype=np.float32)
    q_norm_w = np.asarray(inputs["q_norm_w"], dtype=np.float32)
    k_norm_w = np.asarray(inputs["k_norm_w"], dtype=np.float32)

    if not _is_causal(mask):
        return _reference_numpy(
            x, mask, w_qkv, w_fc, q_norm_w, k_norm_w, inputs.get("subset_attention_size")
        )

    if "nc" not in _CACHE:
        _CACHE["nc"] = _build_nc()
    nc = _CACHE["nc"]

    in_maps = _make_in_maps(x, w_qkv, w_fc, q_norm_w, k_norm_w)
    res = run_bass_kernel_spmd(nc, in_maps, core_ids=list(range(8)))
    parts = [res.results[i]["outp"] for i in range(8)]
    out = np.empty((B, L, D), dtype=np.float32)
    for b in range(B):
        acc = np.zeros((L, D), dtype=np.float64)
        for hg in range(4):
            acc += parts[b * 4 + hg]
        out[b] = acc.astype(np.float32)
    return out
